# revision 7
# baseline (speedup 1.0000x reference)
"""Cross-attention with relative-position-bias MLP on 8 Trainium2 NeuronCores.

Sharding: batch-parallel attention (core c owns batch element c) +
Lq-sharded bias MLP (core c computes bias rows for queries 64c..64c+64),
AllGather of the [512, 12, 512] bias tensor, then full attention per core.

Self-contained: hardcodes all shapes; builds/compiles the Bass kernel on
first call and runs it via bass_utils.run_bass_kernel_spmd on cores 0-7.
"""

import numpy as np

import concourse.bass as bass
import concourse.mybir as mybir
import concourse.tile as tile
from concourse import bacc, bass_utils
from concourse.masks import make_identity

F32 = mybir.dt.float32
AF = mybir.ActivationFunctionType

NCORES = 8
B = 8
L = 512  # LQ == LK
D = 768
H = 12
DH = 64  # head dim
QS = L // NCORES  # 64 queries per core for the bias shard
NCH = D // 128  # 6 chunks of the model dim
SCALE = DH ** -0.5

_CACHE = {}


def _build(dbg=False):
    nc = bacc.Bacc("TRN2", target_bir_lowering=False, debug=False, num_devices=NCORES)

    # ---- DRAM I/O ----
    xqT_d = nc.dram_tensor("xqT", [D, L], F32, kind="ExternalInput")
    kvT_d = nc.dram_tensor("kvT", [D, L], F32, kind="ExternalInput")
    relT_d = nc.dram_tensor("relT", [6, QS * L], F32, kind="ExternalInput")
    WqS_d = nc.dram_tensor("WqS", [128, NCH, D], F32, kind="ExternalInput")
    Wk_d = nc.dram_tensor("Wk", [128, NCH, D], F32, kind="ExternalInput")
    Wv_d = nc.dram_tensor("Wv", [128, NCH, D], F32, kind="ExternalInput")
    Wo_d = nc.dram_tensor("Wo", [128, NCH, D], F32, kind="ExternalInput")
    W1_d = nc.dram_tensor("W1", [6, D], F32, kind="ExternalInput")
    W2_d = nc.dram_tensor("W2", [128, NCH, H], F32, kind="ExternalInput")
    bqS_d = nc.dram_tensor("bqS", [128, NCH], F32, kind="ExternalInput")
    bk_d = nc.dram_tensor("bk", [128, NCH], F32, kind="ExternalInput")
    b1_d = nc.dram_tensor("b1", [128, NCH], F32, kind="ExternalInput")
    b2_d = nc.dram_tensor("b2", [H, 1], F32, kind="ExternalInput")
    bv_d = nc.dram_tensor("bvb", [128, D], F32, kind="ExternalInput")
    bo_d = nc.dram_tensor("bob", [128, D], F32, kind="ExternalInput")
    out_d = nc.dram_tensor("out", [L, D], F32, kind="ExternalOutput")
    if dbg:
        dbg_qT = nc.dram_tensor("dbg_qT", [D, L], F32, kind="ExternalOutput")
        dbg_kT = nc.dram_tensor("dbg_kT", [D, L], F32, kind="ExternalOutput")
        dbg_v = nc.dram_tensor("dbg_v", [L, D], F32, kind="ExternalOutput")
        dbg_attnT = nc.dram_tensor("dbg_attnT", [D, L], F32, kind="ExternalOutput")
        dbg_bshard = nc.dram_tensor("dbg_bshard", [QS * H, L], F32, kind="ExternalOutput")
        dbg_bfull = nc.dram_tensor("dbg_bfull", [L * H, L], F32, kind="ExternalOutput")
        dbg_exp = nc.dram_tensor("dbg_exp", [128, L], F32, kind="ExternalOutput")
        dbg_expT = nc.dram_tensor("dbg_expT", [128, 4 * L], F32, kind="ExternalOutput")

    with tile.TileContext(nc) as tc:
        with (
            tc.tile_pool(name="dram", bufs=1, space="DRAM") as dpool,
            tc.tile_pool(name="const", bufs=1) as cpool,
        ):
            bias_shard = dpool.tile([QS * H, L], F32, name="bias_shard")
            bias_full = dpool.tile(
                [L * H, L], F32, name="bias_full", addr_space="Shared"
            )

            W1_sb = cpool.tile([6, D], F32, name="W1_sb")
            nc.sync.dma_start(W1_sb[:], W1_d[:, :])
            W2_sb = cpool.tile([128, NCH, H], F32, name="W2_sb")
            nc.sync.dma_start(W2_sb[:], W2_d[:, :, :])
            b1_sb = cpool.tile([128, NCH], F32, name="b1_sb")
            nc.sync.dma_start(b1_sb[:], b1_d[:, :])
            b2_sb = cpool.tile([H, 1], F32, name="b2_sb")
            nc.sync.dma_start(b2_sb[:], b2_d[:, :])
            WqS_sb = cpool.tile([128, NCH, D], F32, name="WqS_sb")
            nc.sync.dma_start(WqS_sb[:], WqS_d[:, :, :])
            Wk_sb = cpool.tile([128, NCH, D], F32, name="Wk_sb")
            nc.sync.dma_start(Wk_sb[:], Wk_d[:, :, :])
            Wv_sb = cpool.tile([128, NCH, D], F32, name="Wv_sb")
            nc.sync.dma_start(Wv_sb[:], Wv_d[:, :, :])
            Wo_sb = cpool.tile([128, NCH, D], F32, name="Wo_sb")
            nc.sync.dma_start(Wo_sb[:], Wo_d[:, :, :])
            bq_sb = cpool.tile([128, NCH], F32, name="bq_sb")
            nc.sync.dma_start(bq_sb[:], bqS_d[:, :])
            bk_sb = cpool.tile([128, NCH], F32, name="bk_sb")
            nc.sync.dma_start(bk_sb[:], bk_d[:, :])
            bv_sb = cpool.tile([128, D], F32, name="bv_sb")
            nc.sync.dma_start(bv_sb[:], bv_d[:, :])
            bo_sb = cpool.tile([128, D], F32, name="bo_sb")
            nc.sync.dma_start(bo_sb[:], bo_d[:, :])
            xqT_sb = cpool.tile([128, NCH, L], F32, name="xqT_sb")
            nc.sync.dma_start(
                xqT_sb[:], xqT_d.ap().rearrange("(c p) t -> p c t", p=128)
            )
            kvT_sb = cpool.tile([128, NCH, L], F32, name="kvT_sb")
            nc.sync.dma_start(
                kvT_sb[:], kvT_d.ap().rearrange("(c p) t -> p c t", p=128)
            )
            ident = cpool.tile([128, 128], F32, name="ident")
            make_identity(nc, ident[:])

            # activations that live across phases
            qT_sb = cpool.tile([128, NCH, L], F32, name="qT_sb")
            kT_sb = cpool.tile([128, NCH, L], F32, name="kT_sb")
            v_sb = cpool.tile([128, 4, D], F32, name="v_sb")
            attnT_sb = cpool.tile([128, NCH, L], F32, name="attnT_sb")

            # ---- Phase 1: bias MLP over this core's 64 queries ----
            with (
                tc.tile_pool(name="p1", bufs=3) as p1,
                tc.tile_pool(name="p1gel", bufs=3) as p1gel,
                tc.tile_pool(name="p1out", bufs=2) as p1out,
                tc.tile_pool(name="p1ps", bufs=3, space="PSUM") as p1ps,
                tc.tile_pool(name="p1psb", bufs=2, space="PSUM") as p1psb,
            ):
                for q in range(QS):
                    rel_q = p1.tile([6, L], F32, tag="rel", name=f"rel_{q}")
                    nc.sync.dma_start(rel_q[:], relT_d[:, q * L : (q + 1) * L])
                    bias_ps = p1psb.tile([H, L], F32, tag="biasps", name=f"bps_{q}")
                    for dc in range(NCH):
                        hid = p1ps.tile([128, L], F32, tag="hid", name=f"hid_{q}_{dc}")
                        nc.tensor.matmul(
                            hid[:],
                            W1_sb[:, dc * 128 : (dc + 1) * 128],
                            rel_q[:],
                            start=True,
                            stop=True,
                        )
                        gel = p1gel.tile([128, L], F32, tag="gel", name=f"gel_{q}_{dc}")
                        nc.scalar.activation(
                            gel[:], hid[:], AF.Gelu, bias=b1_sb[:, dc : dc + 1]
                        )
                        nc.tensor.matmul(
                            bias_ps[:],
                            W2_sb[:, dc, :],
                            gel[:],
                            start=(dc == 0),
                            stop=(dc == NCH - 1),
                        )
                    bias_sb = p1out.tile([H, L], F32, tag="biassb", name=f"bsb_{q}")
                    nc.vector.tensor_scalar_add(bias_sb[:], bias_ps[:], b2_sb[:, 0:1])
                    nc.sync.dma_start(bias_shard[q * H : (q + 1) * H, :], bias_sb[:])

            # ---- Phase 2: all-gather the bias across the 8 cores ----
            nc.gpsimd.collective_compute(
                "AllGather",
                mybir.AluOpType.bypass,
                replica_groups=[list(range(NCORES))],
                ins=[bias_shard[:].opt()],
                outs=[bias_full[:].opt()],
            )

            # ---- Phase 3a: q/k/v projections (independent of phases 1-2) ----
            with tc.tile_pool(name="pps", bufs=2, space="PSUM") as pps:
                for oc in range(NCH):
                    ps_q = pps.tile([128, L], F32, tag="psproj", name=f"psq_{oc}")
                    for di in range(NCH):
                        nc.tensor.matmul(
                            ps_q[:],
                            WqS_sb[:, di, oc * 128 : (oc + 1) * 128],
                            xqT_sb[:, di, :],
                            start=(di == 0),
                            stop=(di == NCH - 1),
                        )
                    nc.vector.tensor_scalar_add(
                        qT_sb[:, oc, :], ps_q[:], bq_sb[:, oc : oc + 1]
                    )
                for oc in range(NCH):
                    ps_k = pps.tile([128, L], F32, tag="psproj", name=f"psk_{oc}")
                    for di in range(NCH):
                        nc.tensor.matmul(
                            ps_k[:],
                            Wk_sb[:, di, oc * 128 : (oc + 1) * 128],
                            kvT_sb[:, di, :],
                            start=(di == 0),
                            stop=(di == NCH - 1),
                        )
                    nc.vector.tensor_scalar_add(
                        kT_sb[:, oc, :], ps_k[:], bk_sb[:, oc : oc + 1]
                    )
                for tc4 in range(4):
                    for hf in range(2):
                        ps_v = pps.tile(
                            [128, 384], F32, tag="psv", name=f"psv_{tc4}_{hf}"
                        )
                        for di in range(NCH):
                            nc.tensor.matmul(
                                ps_v[:],
                                kvT_sb[:, di, tc4 * 128 : (tc4 + 1) * 128],
                                Wv_sb[:, di, hf * 384 : (hf + 1) * 384],
                                start=(di == 0),
                                stop=(di == NCH - 1),
                            )
                        nc.vector.tensor_tensor(
                            v_sb[:, tc4, hf * 384 : (hf + 1) * 384],
                            ps_v[:],
                            bv_sb[:, hf * 384 : (hf + 1) * 384],
                            op=mybir.AluOpType.add,
                        )

            # ---- Phase 3b: logits + softmax + AV per head ----
            bias_view = bias_full[:].rearrange("(q h) k -> q h k", h=H)
            with (
                tc.tile_pool(name="lps", bufs=2, space="PSUM") as lps,
                tc.tile_pool(name="trps", bufs=2, space="PSUM") as trps,
                tc.tile_pool(name="avps", bufs=2, space="PSUM") as avps,
                tc.tile_pool(name="battn", bufs=3) as battn,
                tc.tile_pool(name="bexp", bufs=2) as bexp,
                tc.tile_pool(name="bsm", bufs=4) as bsm,
                tc.tile_pool(name="bxp", bufs=2) as bxp,
            ):
                for h in range(H):
                    po = (h % 2) * DH  # partition offset of this head in its chunk
                    ch = h // 2  # which 128-chunk of the model dim
                    expT = bxp.tile([128, 4, L], F32, tag="expT", name=f"expT_{h}")
                    for qc in range(4):
                        ps_l = lps.tile([128, L], F32, tag="logit", name=f"pl_{h}_{qc}")
                        nc.tensor.matmul(
                            ps_l[:],
                            qT_sb[po : po + DH, ch, qc * 128 : (qc + 1) * 128],
                            kT_sb[po : po + DH, ch, :],
                            start=True,
                            stop=False,
                        )
                        bias_t = battn.tile(
                            [128, L], F32, tag="biast", name=f"bt_{h}_{qc}"
                        )
                        nc.sync.dma_start(
                            bias_t[:], bias_view[qc * 128 : (qc + 1) * 128, h, :]
                        )
                        nc.tensor.matmul(
                            ps_l[:], ident[:], bias_t[:], start=False, stop=True
                        )
                        exp_t = bexp.tile([128, L], F32, tag="exp", name=f"ex_{h}_{qc}")
                        sums = bsm.tile([128, 1], F32, tag="sums", name=f"sm_{h}_{qc}")
                        nc.scalar.activation(
                            exp_t[:], ps_l[:], AF.Exp, accum_out=sums[:]
                        )
                        if dbg and h == 0 and qc == 0:
                            nc.sync.dma_start(dbg_exp[:, :], exp_t[:])
                        rc = bsm.tile([128, 1], F32, tag="rc", name=f"rc_{h}_{qc}")
                        nc.vector.reciprocal(rc[:], sums[:])
                        exp_s = bexp.tile(
                            [128, L], F32, tag="exps", name=f"exs_{h}_{qc}"
                        )
                        nc.vector.tensor_scalar_mul(exp_s[:], exp_t[:], rc[:])
                        for kc in range(4):
                            tr = trps.tile(
                                [128, 128], F32, tag="tr", name=f"tr_{h}_{qc}_{kc}"
                            )
                            nc.tensor.transpose(
                                tr[:], exp_s[:, kc * 128 : (kc + 1) * 128], ident[:]
                            )
                            nc.vector.tensor_copy(
                                expT[:, kc, qc * 128 : (qc + 1) * 128], tr[:]
                            )
                    if dbg and h == 0:
                        nc.sync.dma_start(
                            dbg_expT.ap().rearrange("p (c t) -> p c t", c=4), expT[:]
                        )
                    ps_av = avps.tile([DH, L], F32, tag="av", name=f"av_{h}")
                    for kc in range(4):
                        nc.tensor.matmul(
                            ps_av[:],
                            v_sb[:, kc, h * DH : (h + 1) * DH],
                            expT[:, kc, :],
                            start=(kc == 0),
                            stop=(kc == 3),
                        )
                    nc.vector.tensor_copy(attnT_sb[po : po + DH, ch, :], ps_av[:])

                # ---- Phase 3c: output projection ----
                with tc.tile_pool(name="ops", bufs=2, space="PSUM") as ops:
                    for tc4 in range(4):
                        out_sb = battn.tile(
                            [128, D], F32, tag="osb", name=f"osb_{tc4}"
                        )
                        for hf in range(2):
                            ps_o = ops.tile(
                                [128, 384], F32, tag="pso", name=f"pso_{tc4}_{hf}"
                            )
                            for dc in range(NCH):
                                nc.tensor.matmul(
                                    ps_o[:],
                                    attnT_sb[:, dc, tc4 * 128 : (tc4 + 1) * 128],
                                    Wo_sb[:, dc, hf * 384 : (hf + 1) * 384],
                                    start=(dc == 0),
                                    stop=(dc == NCH - 1),
                                )
                            nc.vector.tensor_tensor(
                                out_sb[:, hf * 384 : (hf + 1) * 384],
                                ps_o[:],
                                bo_sb[:, hf * 384 : (hf + 1) * 384],
                                op=mybir.AluOpType.add,
                            )
                        nc.sync.dma_start(
                            out_d[tc4 * 128 : (tc4 + 1) * 128, :], out_sb[:]
                        )

                if dbg:
                    nc.sync.dma_start(
                        dbg_qT.ap().rearrange("(c p) t -> p c t", p=128), qT_sb[:]
                    )
                    nc.sync.dma_start(
                        dbg_kT.ap().rearrange("(c p) t -> p c t", p=128), kT_sb[:]
                    )
                    nc.sync.dma_start(
                        dbg_v.ap().rearrange("(c p) t -> p c t", p=128), v_sb[:]
                    )
                    nc.sync.dma_start(
                        dbg_attnT.ap().rearrange("(c p) t -> p c t", p=128), attnT_sb[:]
                    )
                    nc.sync.dma_start(dbg_bshard[:, :], bias_shard[:])
                    nc.sync.dma_start(dbg_bfull[:, :], bias_full[:])

    nc.compile()
    return nc


def _get_nc():
    if "nc" not in _CACHE:
        _CACHE["nc"] = _build()
    return _CACHE["nc"]


def kernel(
    query,
    key_value,
    query_coords,
    key_coords,
    Wq,
    bq,
    Wk,
    bk,
    Wv,
    bv,
    Wo,
    bo,
    W1,
    b1,
    W2,
    b2,
):
    query = np.asarray(query, np.float32)
    key_value = np.asarray(key_value, np.float32)
    query_coords = np.asarray(query_coords, np.float32)
    key_coords = np.asarray(key_coords, np.float32)

    def chunked(w):  # [768, X] -> [128, 6, X]
        w = np.asarray(w, np.float32)
        return np.ascontiguousarray(w.reshape(NCH, 128, -1).transpose(1, 0, 2))

    def pchunk(b):  # [768] -> [128, 6]
        return np.ascontiguousarray(np.asarray(b, np.float32).reshape(NCH, 128).T)

    WqS = chunked(np.asarray(Wq, np.float32) * np.float32(SCALE))
    Wk_l = chunked(Wk)
    Wv_l = chunked(Wv)
    Wo_l = chunked(Wo)
    W2_l = chunked(W2)
    W1_l = np.ascontiguousarray(np.asarray(W1, np.float32))
    bqS = pchunk(np.asarray(bq, np.float32) * np.float32(SCALE))
    bk_l = pchunk(bk)
    b1_l = pchunk(b1)
    b2_l = np.ascontiguousarray(np.asarray(b2, np.float32).reshape(H, 1))
    bv_b = np.ascontiguousarray(
        np.broadcast_to(np.asarray(bv, np.float32), (128, D))
    )
    bo_b = np.ascontiguousarray(
        np.broadcast_to(np.asarray(bo, np.float32), (128, D))
    )

    in_maps = []
    for c in range(NCORES):
        qs = slice(c * QS, (c + 1) * QS)
        delta = query_coords[qs, None, :] - key_coords[None, :, :]  # [QS, L, 2]
        rel = np.concatenate([delta, np.abs(delta), np.square(delta)], axis=-1)
        relT = np.ascontiguousarray(
            rel.reshape(QS * L, 6).T
        )  # [6, QS*L], pair index = q*L + k
        in_maps.append(
            {
                "xqT": np.ascontiguousarray(query[c].T),
                "kvT": np.ascontiguousarray(key_value[c].T),
                "relT": relT,
                "WqS": WqS,
                "Wk": Wk_l,
                "Wv": Wv_l,
                "Wo": Wo_l,
                "W1": W1_l,
                "W2": W2_l,
                "bqS": bqS,
                "bk": bk_l,
                "b1": b1_l,
                "b2": b2_l,
                "bvb": bv_b,
                "bob": bo_b,
            }
        )

    nc = _get_nc()
    res = bass_utils.run_bass_kernel_spmd(nc, in_maps, core_ids=list(range(NCORES)))
    out = np.stack([res.results[c]["out"] for c in range(NCORES)], axis=0)
    return out.astype(np.float32)


# revision 13
# speedup vs baseline: 1.2018x; 1.2018x over previous
"""Cross-attention with relative-position-bias MLP on 8 Trainium2 NeuronCores.

Sharding: batch-parallel attention (core c owns batch element c) +
Lq-sharded bias MLP (core c computes bias rows for queries 64c..64c+64),
AllGather of the [512, 12, 512] bias tensor, then full attention per core.

Precision strategy (PE fp32 matmul is 4-8x slower than 16-bit):
- bias MLP mm1: bf16 hi/lo split packed into K=128 (exact to ~2^-17)
- bias MLP mm2: fp16 hidden + fp16 W2 (error ~4e-4 absolute on a +-25 bias)
- QK logits: fp16 hi/lo split of q (packed K=128) + qhi*klo term
- AV: fp16 v_hi x (exp hi + lo) split
- O projection: fp16 3-term hi/lo split
- q/k/v projections: fp32 (exact)

Self-contained: hardcodes all shapes; builds/compiles the Bass kernel on
first call and runs it via bass_utils.run_bass_kernel_spmd on cores 0-7.
"""

import numpy as np

import concourse.bass as bass
import concourse.mybir as mybir
import concourse.tile as tile
from concourse import bacc, bass_utils
from concourse.masks import make_identity

F32 = mybir.dt.float32
BF16 = mybir.dt.bfloat16
FP16 = mybir.dt.float16
AF = mybir.ActivationFunctionType
ADD = mybir.AluOpType.add
SUB = mybir.AluOpType.subtract

NCORES = 8
B = 8
L = 512
D = 768
H = 12
DH = 64
QS = L // NCORES  # 64 queries per core in the bias shard
NCH = D // 128  # 6 chunks of the model dim
SCALE = DH ** -0.5

_CACHE = {}


def _build(dbg=False):
    nc = bacc.Bacc("TRN2", target_bir_lowering=False, debug=False, num_devices=NCORES)

    # ---- DRAM I/O ----
    xqT_d = nc.dram_tensor("xqT", [D, L], F32, kind="ExternalInput")
    kvT_d = nc.dram_tensor("kvT", [D, L], F32, kind="ExternalInput")
    relP_d = nc.dram_tensor("relP", [128, QS * L], BF16, kind="ExternalInput")
    WqS_d = nc.dram_tensor("WqS", [128, NCH, D], F32, kind="ExternalInput")
    Wk_d = nc.dram_tensor("Wk", [128, NCH, D], F32, kind="ExternalInput")
    Wv_d = nc.dram_tensor("Wv", [128, NCH, D], F32, kind="ExternalInput")
    WoH_d = nc.dram_tensor("WoH", [128, NCH, D], FP16, kind="ExternalInput")
    WoL_d = nc.dram_tensor("WoL", [128, NCH, D], FP16, kind="ExternalInput")
    W1P_d = nc.dram_tensor("W1P", [128, D], BF16, kind="ExternalInput")
    W2h_d = nc.dram_tensor("W2h", [128, NCH, H], FP16, kind="ExternalInput")
    bqS_d = nc.dram_tensor("bqS", [128, NCH], F32, kind="ExternalInput")
    bk_d = nc.dram_tensor("bk", [128, NCH], F32, kind="ExternalInput")
    b1_d = nc.dram_tensor("b1", [128, NCH], F32, kind="ExternalInput")
    b2_d = nc.dram_tensor("b2", [H, 1], F32, kind="ExternalInput")
    bv_d = nc.dram_tensor("bvb", [128, D], F32, kind="ExternalInput")
    bo_d = nc.dram_tensor("bob", [128, D], F32, kind="ExternalInput")
    out_d = nc.dram_tensor("out", [L, D], F32, kind="ExternalOutput")
    if dbg:
        dbg_bfull = nc.dram_tensor("dbg_bfull", [L * H, L], F32, kind="ExternalOutput")

    with tile.TileContext(nc) as tc:
        with (
            tc.tile_pool(name="dram", bufs=1, space="DRAM") as dpool,
            tc.tile_pool(name="persist", bufs=1) as pp,
        ):
            bias_shard = dpool.tile([QS * H, L], F32, name="bias_shard")
            bias_full = dpool.tile(
                [L * H, L], F32, name="bias_full", addr_space="Shared"
            )

            W1p_sb = pp.tile([128, D], BF16, name="W1p_sb")
            nc.sync.dma_start(W1p_sb[:], W1P_d[:, :])
            W2h_sb = pp.tile([128, NCH, H], FP16, name="W2h_sb")
            nc.sync.dma_start(W2h_sb[:], W2h_d[:, :, :])
            WoH_sb = pp.tile([128, NCH, D], FP16, name="WoH_sb")
            nc.sync.dma_start(WoH_sb[:], WoH_d[:, :, :])
            WoL_sb = pp.tile([128, NCH, D], FP16, name="WoL_sb")
            nc.sync.dma_start(WoL_sb[:], WoL_d[:, :, :])
            b1_sb = pp.tile([128, NCH], F32, name="b1_sb")
            nc.sync.dma_start(b1_sb[:], b1_d[:, :])
            b2_sb = pp.tile([H, 1], F32, name="b2_sb")
            nc.sync.dma_start(b2_sb[:], b2_d[:, :])
            bq_sb = pp.tile([128, NCH], F32, name="bq_sb")
            nc.sync.dma_start(bq_sb[:], bqS_d[:, :])
            bk_sb = pp.tile([128, NCH], F32, name="bk_sb")
            nc.sync.dma_start(bk_sb[:], bk_d[:, :])
            bv_sb = pp.tile([128, D], F32, name="bv_sb")
            nc.sync.dma_start(bv_sb[:], bv_d[:, :])
            bo_sb = pp.tile([128, D], F32, name="bo_sb")
            nc.sync.dma_start(bo_sb[:], bo_d[:, :])
            ident = pp.tile([128, 128], F32, name="ident")
            make_identity(nc, ident[:])

            # split activations living across phases (fp16 hi/lo, natural layout)
            qhi = pp.tile([128, NCH, L], FP16, name="qhi")
            qlo = pp.tile([128, NCH, L], FP16, name="qlo")
            khi = pp.tile([128, NCH, L], FP16, name="khi")
            klo = pp.tile([128, NCH, L], FP16, name="klo")
            vhi = pp.tile([128, 4, D], FP16, name="vhi")
            aTh = pp.tile([128, NCH, L], FP16, name="aTh")
            aTl = pp.tile([128, NCH, L], FP16, name="aTl")

            # ---- Phase 1: bias MLP over this core's 64 queries (2q per step) ----
            with (
                tc.tile_pool(name="p1rel", bufs=3) as p1rel,
                tc.tile_pool(name="p1gel", bufs=3) as p1gel,
                tc.tile_pool(name="p1out", bufs=3) as p1out,
                tc.tile_pool(name="p1ps", bufs=2, space="PSUM") as p1ps,
                tc.tile_pool(name="p1psb", bufs=3, space="PSUM") as p1psb,
            ):
                for qq in range(QS // 2):
                    rel2 = p1rel.tile([128, 2 * L], BF16, tag="rel", name=f"rel_{qq}")
                    nc.sync.dma_start(
                        rel2[:], relP_d[:, qq * 2 * L : (qq + 1) * 2 * L]
                    )
                    bps = [
                        p1psb.tile([H, L], F32, tag="bps", name=f"bps_{qq}_{j}")
                        for j in range(2)
                    ]
                    for dc in range(NCH):
                        hidw = p1ps.tile(
                            [128, 2 * L], F32, tag="hid", name=f"hid_{qq}_{dc}"
                        )
                        for j in range(2):
                            nc.tensor.matmul(
                                hidw[:, j * L : (j + 1) * L],
                                W1p_sb[:, dc * 128 : (dc + 1) * 128],
                                rel2[:, j * L : (j + 1) * L],
                                start=True,
                                stop=True,
                            )
                        gelw = p1gel.tile(
                            [128, 2 * L], FP16, tag="gel", name=f"gel_{qq}_{dc}"
                        )
                        nc.scalar.activation(
                            gelw[:], hidw[:], AF.Gelu, bias=b1_sb[:, dc : dc + 1]
                        )
                        for j in range(2):
                            nc.tensor.matmul(
                                bps[j][:],
                                W2h_sb[:, dc, :],
                                gelw[:, j * L : (j + 1) * L],
                                start=(dc == 0),
                                stop=(dc == NCH - 1),
                            )
                    for j in range(2):
                        q = qq * 2 + j
                        bsb = p1out.tile([H, L], F32, tag="bsb", name=f"bsb_{q}")
                        nc.vector.tensor_scalar_add(bsb[:], bps[j][:], b2_sb[:, 0:1])
                        nc.sync.dma_start(bias_shard[q * H : (q + 1) * H, :], bsb[:])

            # ---- Phase 2: all-gather the bias across the 8 cores ----
            nc.gpsimd.collective_compute(
                "AllGather",
                mybir.AluOpType.bypass,
                replica_groups=[list(range(NCORES))],
                ins=[bias_shard[:].opt()],
                outs=[bias_full[:].opt()],
            )
            if dbg:
                nc.sync.dma_start(dbg_bfull[:, :], bias_full[:])

            # ---- Phase 3a: q/k/v projections (fp32, exact) ----
            with (
                tc.tile_pool(name="wpool", bufs=1) as wp,
                tc.tile_pool(name="ptmp", bufs=3) as ptmp,
                tc.tile_pool(name="pps", bufs=2, space="PSUM") as pps,
            ):
                WqS_sb = wp.tile([128, NCH, D], F32, name="WqS_sb")
                nc.sync.dma_start(WqS_sb[:], WqS_d[:, :, :])
                Wk_sb = wp.tile([128, NCH, D], F32, name="Wk_sb")
                nc.sync.dma_start(Wk_sb[:], Wk_d[:, :, :])
                Wv_sb = wp.tile([128, NCH, D], F32, name="Wv_sb")
                nc.sync.dma_start(Wv_sb[:], Wv_d[:, :, :])
                xqT_sb = wp.tile([128, NCH, L], F32, name="xqT_sb")
                nc.sync.dma_start(
                    xqT_sb[:], xqT_d.ap().rearrange("(c p) t -> p c t", p=128)
                )
                kvT_sb = wp.tile([128, NCH, L], F32, name="kvT_sb")
                nc.sync.dma_start(
                    kvT_sb[:], kvT_d.ap().rearrange("(c p) t -> p c t", p=128)
                )

                def proj_split(W_sb, x_sb, b_sb, hi_t, lo_t, pfx):
                    for oc in range(NCH):
                        ps = pps.tile([128, L], F32, tag="psp", name=f"pp{pfx}_{oc}")
                        for di in range(NCH):
                            nc.tensor.matmul(
                                ps[:],
                                W_sb[:, di, oc * 128 : (oc + 1) * 128],
                                x_sb[:, di, :],
                                start=(di == 0),
                                stop=(di == NCH - 1),
                            )
                        f32t = ptmp.tile([128, L], F32, tag="f32t", name=f"pt{pfx}_{oc}")
                        nc.vector.tensor_scalar_add(f32t[:], ps[:], b_sb[:, oc : oc + 1])
                        nc.vector.tensor_copy(hi_t[:, oc, :], f32t[:])
                        nc.vector.tensor_tensor(
                            lo_t[:, oc, :], f32t[:], hi_t[:, oc, :], op=SUB
                        )

                proj_split(WqS_sb, xqT_sb, bq_sb, qhi, qlo, "q")
                proj_split(Wk_sb, kvT_sb, bk_sb, khi, klo, "k")
                # v: natural layout [t, dv] in fp16 (hi only; lo error ~5e-4*|v| is
                # attenuated by softmax-weight averaging)
                for tc4 in range(4):
                    for hf in range(2):
                        ps = pps.tile(
                            [128, 384], F32, tag="psv", name=f"ppv_{tc4}_{hf}"
                        )
                        for di in range(NCH):
                            nc.tensor.matmul(
                                ps[:],
                                kvT_sb[:, di, tc4 * 128 : (tc4 + 1) * 128],
                                Wv_sb[:, di, hf * 384 : (hf + 1) * 384],
                                start=(di == 0),
                                stop=(di == NCH - 1),
                            )
                        nc.vector.tensor_tensor(
                            vhi[:, tc4, hf * 384 : (hf + 1) * 384],
                            ps[:],
                            bv_sb[:, hf * 384 : (hf + 1) * 384],
                            op=ADD,
                        )

            # ---- Phase 3b: logits + softmax + AV per head ----
            bias_view = bias_full[:].rearrange("(q h) k -> q h k", h=H)
            with (
                tc.tile_pool(name="lps", bufs=2, space="PSUM") as lps,
                tc.tile_pool(name="trps", bufs=2, space="PSUM") as trps,
                tc.tile_pool(name="avps", bufs=2, space="PSUM") as avps,
                tc.tile_pool(name="battn", bufs=3) as battn,
                tc.tile_pool(name="bexp", bufs=2) as bexp,
                tc.tile_pool(name="bsm", bufs=4) as bsm,
                tc.tile_pool(name="bxp", bufs=2) as bxp,
            ):
                for h in range(H):
                    po = (h % 2) * DH
                    ch = h // 2
                    eTh = bxp.tile([128, 4, L], FP16, tag="eTh", name=f"eTh_{h}")
                    eTl = bxp.tile([128, 4, L], FP16, tag="eTl", name=f"eTl_{h}")
                    hs = slice(po, po + DH)
                    for qc in range(4):
                        cs = slice(qc * 128, (qc + 1) * 128)
                        ps_l = lps.tile([128, L], F32, tag="lg", name=f"pl_{h}_{qc}")
                        # q.k = qhi.khi + qlo.khi + qhi.klo (qlo.klo ~ 2^-22)
                        nc.tensor.matmul(
                            ps_l[:], qhi[hs, ch, cs], khi[hs, ch, :],
                            start=True, stop=False,
                        )
                        nc.tensor.matmul(
                            ps_l[:], qlo[hs, ch, cs], khi[hs, ch, :],
                            start=False, stop=False,
                        )
                        nc.tensor.matmul(
                            ps_l[:], qhi[hs, ch, cs], klo[hs, ch, :],
                            start=False, stop=True,
                        )
                        bias_t = battn.tile(
                            [128, L], F32, tag="biast", name=f"bt_{h}_{qc}"
                        )
                        nc.sync.dma_start(
                            bias_t[:], bias_view[qc * 128 : (qc + 1) * 128, h, :]
                        )
                        lsb = battn.tile([128, L], F32, tag="lsb", name=f"ls_{h}_{qc}")
                        nc.vector.tensor_tensor(lsb[:], ps_l[:], bias_t[:], op=ADD)
                        exp_t = bexp.tile([128, L], F32, tag="exp", name=f"ex_{h}_{qc}")
                        sums = bsm.tile([128, 1], F32, tag="sums", name=f"sm_{h}_{qc}")
                        nc.scalar.activation(
                            exp_t[:], lsb[:], AF.Exp, accum_out=sums[:]
                        )
                        rc = bsm.tile([128, 1], F32, tag="rc", name=f"rc_{h}_{qc}")
                        nc.vector.reciprocal(rc[:], sums[:])
                        exp_s = bexp.tile(
                            [128, L], F32, tag="exps", name=f"exs_{h}_{qc}"
                        )
                        nc.gpsimd.tensor_scalar_mul(exp_s[:], exp_t[:], rc[:])
                        for kc in range(4):
                            tr = trps.tile(
                                [128, 128], F32, tag="tr", name=f"tr_{h}_{qc}_{kc}"
                            )
                            nc.tensor.transpose(
                                tr[:], exp_s[:, kc * 128 : (kc + 1) * 128], ident[:]
                            )
                            nc.vector.tensor_copy(
                                eTh[:, kc, qc * 128 : (qc + 1) * 128], tr[:]
                            )
                            nc.vector.tensor_tensor(
                                eTl[:, kc, qc * 128 : (qc + 1) * 128],
                                tr[:],
                                eTh[:, kc, qc * 128 : (qc + 1) * 128],
                                op=SUB,
                            )
                    ps_av = avps.tile([128, L], F32, tag="av", name=f"av_{h}")
                    for kc in range(4):
                        nc.tensor.matmul(
                            ps_av[hs, :],
                            vhi[:, kc, h * DH : (h + 1) * DH],
                            eTh[:, kc, :],
                            start=(kc == 0),
                            stop=False,
                        )
                        nc.tensor.matmul(
                            ps_av[hs, :],
                            vhi[:, kc, h * DH : (h + 1) * DH],
                            eTl[:, kc, :],
                            start=False,
                            stop=(kc == 3),
                        )
                    nc.vector.tensor_copy(aTh[hs, ch, :], ps_av[hs, :])
                    nc.vector.tensor_tensor(
                        aTl[hs, ch, :], ps_av[hs, :], aTh[hs, ch, :], op=SUB
                    )

                # ---- Phase 3c: output projection (fp16 3-term split) ----
                with tc.tile_pool(name="ops", bufs=2, space="PSUM") as ops:
                    for tc4 in range(4):
                        out_sb = battn.tile([128, D], F32, tag="osb", name=f"osb_{tc4}")
                        for hf in range(2):
                            ps_o = ops.tile(
                                [128, 384], F32, tag="pso", name=f"pso_{tc4}_{hf}"
                            )
                            sl = slice(hf * 384, (hf + 1) * 384)
                            for dc in range(NCH):
                                ts4 = slice(tc4 * 128, (tc4 + 1) * 128)
                                nc.tensor.matmul(
                                    ps_o[:], aTh[:, dc, ts4], WoH_sb[:, dc, sl],
                                    start=(dc == 0), stop=False,
                                )
                                nc.tensor.matmul(
                                    ps_o[:], aTl[:, dc, ts4], WoH_sb[:, dc, sl],
                                    start=False, stop=False,
                                )
                                nc.tensor.matmul(
                                    ps_o[:], aTh[:, dc, ts4], WoL_sb[:, dc, sl],
                                    start=False, stop=(dc == NCH - 1),
                                )
                            nc.vector.tensor_tensor(
                                out_sb[:, sl], ps_o[:], bo_sb[:, sl], op=ADD
                            )
                        nc.sync.dma_start(
                            out_d[tc4 * 128 : (tc4 + 1) * 128, :], out_sb[:]
                        )

    nc.compile()
    return nc


def _get_nc():
    if "nc" not in _CACHE:
        _CACHE["nc"] = _build()
    return _CACHE["nc"]


def _hi_lo(a, dt):
    hi = a.astype(dt)
    lo = (a - hi.astype(np.float32)).astype(dt)
    return hi, lo


def kernel(
    query,
    key_value,
    query_coords,
    key_coords,
    Wq,
    bq,
    Wk,
    bk,
    Wv,
    bv,
    Wo,
    bo,
    W1,
    b1,
    W2,
    b2,
):
    query = np.asarray(query, np.float32)
    key_value = np.asarray(key_value, np.float32)
    query_coords = np.asarray(query_coords, np.float32)
    key_coords = np.asarray(key_coords, np.float32)

    def chunked(w, dt=np.float32):  # [768, X] -> [128, 6, X]
        w = np.asarray(w, dt)
        return np.ascontiguousarray(w.reshape(NCH, 128, -1).transpose(1, 0, 2))

    def pchunk(b):  # [768] -> [128, 6]
        return np.ascontiguousarray(np.asarray(b, np.float32).reshape(NCH, 128).T)

    WqS = chunked(np.asarray(Wq, np.float32) * np.float32(SCALE))
    Wk_l = chunked(Wk)
    Wv_l = chunked(Wv)
    WoHf, WoLf = _hi_lo(np.asarray(Wo, np.float32), np.float16)
    WoH_l = chunked(WoHf, np.float16)
    WoL_l = chunked(WoLf, np.float16)
    W2h_l = chunked(np.asarray(W2, np.float32).astype(np.float16), np.float16)
    # W1 packed: rows [W1hi; W1lo; W1hi; W1lo; zeros(104)] in bf16
    import ml_dtypes

    W1f = np.asarray(W1, np.float32)
    W1hi, W1lo = _hi_lo(W1f, ml_dtypes.bfloat16)
    W1P = np.zeros((128, D), ml_dtypes.bfloat16)
    W1P[0:6] = W1hi
    W1P[6:12] = W1hi
    W1P[12:18] = W1lo
    W1P[18:24] = W1lo
    bqS = pchunk(np.asarray(bq, np.float32) * np.float32(SCALE))
    bk_l = pchunk(bk)
    b1_l = pchunk(b1)
    b2_l = np.ascontiguousarray(np.asarray(b2, np.float32).reshape(H, 1))
    bv_b = np.ascontiguousarray(np.broadcast_to(np.asarray(bv, np.float32), (128, D)))
    bo_b = np.ascontiguousarray(np.broadcast_to(np.asarray(bo, np.float32), (128, D)))

    in_maps = []
    for c in range(NCORES):
        qs = slice(c * QS, (c + 1) * QS)
        delta = query_coords[qs, None, :] - key_coords[None, :, :]  # [QS, L, 2]
        rel = np.concatenate([delta, np.abs(delta), np.square(delta)], axis=-1)
        relT = rel.reshape(QS * L, 6).T  # [6, QS*L], pair index = q*L + k
        rhi, rlo = _hi_lo(relT, ml_dtypes.bfloat16)
        relP = np.zeros((128, QS * L), ml_dtypes.bfloat16)
        relP[0:6] = rhi
        relP[6:12] = rlo
        relP[12:18] = rhi
        relP[18:24] = rlo
        in_maps.append(
            {
                "xqT": np.ascontiguousarray(query[c].T),
                "kvT": np.ascontiguousarray(key_value[c].T),
                "relP": relP,
                "WqS": WqS,
                "Wk": Wk_l,
                "Wv": Wv_l,
                "WoH": WoH_l,
                "WoL": WoL_l,
                "W1P": W1P,
                "W2h": W2h_l,
                "bqS": bqS,
                "bk": bk_l,
                "b1": b1_l,
                "b2": b2_l,
                "bvb": bv_b,
                "bob": bo_b,
            }
        )

    nc = _get_nc()
    res = bass_utils.run_bass_kernel_spmd(nc, in_maps, core_ids=list(range(NCORES)))
    out = np.stack([res.results[c]["out"] for c in range(NCORES)], axis=0)
    return out.astype(np.float32)


# revision 16
# speedup vs baseline: 1.7165x; 1.4283x over previous
"""Cross-attention with relative-position-bias MLP on 8 Trainium2 NeuronCores.

Sharding: batch-parallel attention (core c owns batch element c) +
Lq-sharded bias MLP (core c computes bias rows for queries 64c..64c+64),
AllGather of the [512, 12, 512] bias tensor, then full attention per core.

Precision strategy (PE fp32 matmul is 4-8x slower than 16-bit / f32r):
- bias MLP mm1: bf16 hi/lo split packed into K=128 (exact to ~2^-17)
- bias MLP mm2: fp16 hidden x (W2hi + W2lo fp16 split, accumulated in PSUM)
- projections / QK / AV / O: f32r (TF32-class, ~1.5e-4) via AP bitcast
- softmax: fp32 exp with fused row-sum, fp32 transposes

Self-contained: hardcodes all shapes; builds/compiles the Bass kernel on
first call and runs it via bass_utils.run_bass_kernel_spmd on cores 0-7.
"""

import numpy as np

import concourse.bass as bass
import concourse.mybir as mybir
import concourse.tile as tile
from concourse import bacc, bass_utils
from concourse.masks import make_identity

F32 = mybir.dt.float32
F32R = mybir.dt.float32r
BF16 = mybir.dt.bfloat16
FP16 = mybir.dt.float16
AF = mybir.ActivationFunctionType
ADD = mybir.AluOpType.add

NCORES = 8
B = 8
L = 512
D = 768
H = 12
DH = 64
QS = L // NCORES
NCH = D // 128
SCALE = DH ** -0.5

_CACHE = {}


def _build(dbg=False):
    nc = bacc.Bacc("TRN2", target_bir_lowering=False, debug=False, num_devices=NCORES)

    xqT_d = nc.dram_tensor("xqT", [D, L], F32R, kind="ExternalInput")
    kvT_d = nc.dram_tensor("kvT", [D, L], F32R, kind="ExternalInput")
    relP_d = nc.dram_tensor("relP", [128, QS * L], BF16, kind="ExternalInput")
    WqS_d = nc.dram_tensor("WqS", [128, NCH, D], F32R, kind="ExternalInput")
    Wk_d = nc.dram_tensor("Wk", [128, NCH, D], F32R, kind="ExternalInput")
    Wv_d = nc.dram_tensor("Wv", [128, NCH, D], F32R, kind="ExternalInput")
    Wo_d = nc.dram_tensor("Wo", [DH, H, D], F32R, kind="ExternalInput")
    W1P_d = nc.dram_tensor("W1P", [128, D], BF16, kind="ExternalInput")
    W2h_d = nc.dram_tensor("W2h", [128, NCH, H], FP16, kind="ExternalInput")
    W2l_d = nc.dram_tensor("W2l", [128, NCH, H], FP16, kind="ExternalInput")
    bqS_d = nc.dram_tensor("bqS", [128, NCH], F32, kind="ExternalInput")
    bk_d = nc.dram_tensor("bk", [128, NCH], F32, kind="ExternalInput")
    b1_d = nc.dram_tensor("b1", [128, NCH], F32, kind="ExternalInput")
    b2_d = nc.dram_tensor("b2", [H, 1], F32, kind="ExternalInput")
    bv_d = nc.dram_tensor("bvb", [128, D], F32, kind="ExternalInput")
    bo_d = nc.dram_tensor("bob", [128, D], F32, kind="ExternalInput")
    out_d = nc.dram_tensor("out", [L, D], F32, kind="ExternalOutput")
    if dbg:
        dbg_bfull = nc.dram_tensor("dbg_bfull", [L * H, L], F32, kind="ExternalOutput")

    with tile.TileContext(nc) as tc:
        with (
            tc.tile_pool(name="dram", bufs=1, space="DRAM") as dpool,
            tc.tile_pool(name="persist", bufs=1) as pp,
        ):
            bias_shard = dpool.tile([QS * H, L], F32, name="bias_shard")
            bias_full = dpool.tile(
                [L * H, L], F32, name="bias_full", addr_space="Shared"
            )

            W1p_sb = pp.tile([128, D], BF16, name="W1p_sb")
            nc.sync.dma_start(W1p_sb[:], W1P_d[:, :])
            W2h_sb = pp.tile([128, NCH, H], FP16, name="W2h_sb")
            nc.sync.dma_start(W2h_sb[:], W2h_d[:, :, :])
            W2l_sb = pp.tile([128, NCH, H], FP16, name="W2l_sb")
            nc.sync.dma_start(W2l_sb[:], W2l_d[:, :, :])
            Wo_sb = pp.tile([DH, H, D], F32R, name="Wo_sb")
            nc.sync.dma_start(Wo_sb[:], Wo_d[:, :, :])
            b1_sb = pp.tile([128, NCH], F32, name="b1_sb")
            nc.sync.dma_start(b1_sb[:], b1_d[:, :])
            b2_sb = pp.tile([H, 1], F32, name="b2_sb")
            nc.sync.dma_start(b2_sb[:], b2_d[:, :])
            bq_sb = pp.tile([128, NCH], F32, name="bq_sb")
            nc.sync.dma_start(bq_sb[:], bqS_d[:, :])
            bk_sb = pp.tile([128, NCH], F32, name="bk_sb")
            nc.sync.dma_start(bk_sb[:], bk_d[:, :])
            bv_sb = pp.tile([128, D], F32, name="bv_sb")
            nc.sync.dma_start(bv_sb[:], bv_d[:, :])
            bo_sb = pp.tile([128, D], F32, name="bo_sb")
            nc.sync.dma_start(bo_sb[:], bo_d[:, :])
            ident = pp.tile([128, 128], F32, name="ident")
            make_identity(nc, ident[:])

            qT_sb = pp.tile([128, NCH, L], F32R, name="qT_sb")
            kT_sb = pp.tile([128, NCH, L], F32R, name="kT_sb")
            v_sb = pp.tile([128, 4, D], F32R, name="v_sb")
            attnT = pp.tile([DH, H, L], F32R, name="attnT")

            # ---- Phase 1: bias MLP over this core's 64 queries (2q per step) ----
            with (
                tc.tile_pool(name="p1rel", bufs=3) as p1rel,
                tc.tile_pool(name="p1gel", bufs=3) as p1gel,
                tc.tile_pool(name="p1out", bufs=3) as p1out,
                tc.tile_pool(name="p1ps", bufs=2, space="PSUM") as p1ps,
                tc.tile_pool(name="p1psb", bufs=3, space="PSUM") as p1psb,
            ):
                for qq in range(QS // 2):
                    rel2 = p1rel.tile([128, 2 * L], BF16, tag="rel", name=f"rel_{qq}")
                    nc.sync.dma_start(
                        rel2[:], relP_d[:, qq * 2 * L : (qq + 1) * 2 * L]
                    )
                    bps = [
                        p1psb.tile([H, L], F32, tag="bps", name=f"bps_{qq}_{j}")
                        for j in range(2)
                    ]
                    for dc in range(NCH):
                        hidw = p1ps.tile(
                            [128, 2 * L], F32, tag="hid", name=f"hid_{qq}_{dc}"
                        )
                        for j in range(2):
                            nc.tensor.matmul(
                                hidw[:, j * L : (j + 1) * L],
                                W1p_sb[:, dc * 128 : (dc + 1) * 128],
                                rel2[:, j * L : (j + 1) * L],
                                start=True,
                                stop=True,
                            )
                        gelw = p1gel.tile(
                            [128, 2 * L], FP16, tag="gel", name=f"gel_{qq}_{dc}"
                        )
                        nc.scalar.activation(
                            gelw[:], hidw[:], AF.Gelu, bias=b1_sb[:, dc : dc + 1]
                        )
                        for j in range(2):
                            nc.tensor.matmul(
                                bps[j][:],
                                W2h_sb[:, dc, :],
                                gelw[:, j * L : (j + 1) * L],
                                start=(dc == 0),
                                stop=False,
                            )
                            nc.tensor.matmul(
                                bps[j][:],
                                W2l_sb[:, dc, :],
                                gelw[:, j * L : (j + 1) * L],
                                start=False,
                                stop=(dc == NCH - 1),
                            )
                    for j in range(2):
                        q = qq * 2 + j
                        bsb = p1out.tile([H, L], F32, tag="bsb", name=f"bsb_{q}")
                        nc.vector.tensor_scalar_add(bsb[:], bps[j][:], b2_sb[:, 0:1])
                        nc.sync.dma_start(bias_shard[q * H : (q + 1) * H, :], bsb[:])

            # ---- Phase 3a: q/k/v projections (f32r, overlaps the all-gather) ----
            with (
                tc.tile_pool(name="wpool", bufs=1) as wp,
                tc.tile_pool(name="ptmp", bufs=3) as ptmp,
                tc.tile_pool(name="pps", bufs=2, space="PSUM") as pps,
            ):
                WqS_sb = wp.tile([128, NCH, D], F32R, name="WqS_sb")
                nc.sync.dma_start(WqS_sb[:], WqS_d[:, :, :])
                Wk_sb = wp.tile([128, NCH, D], F32R, name="Wk_sb")
                nc.sync.dma_start(Wk_sb[:], Wk_d[:, :, :])
                Wv_sb = wp.tile([128, NCH, D], F32R, name="Wv_sb")
                nc.sync.dma_start(Wv_sb[:], Wv_d[:, :, :])
                xqT_sb = wp.tile([128, NCH, L], F32R, name="xqT_sb")
                nc.sync.dma_start(
                    xqT_sb[:], xqT_d.ap().rearrange("(c p) t -> p c t", p=128)
                )
                kvT_sb = wp.tile([128, NCH, L], F32R, name="kvT_sb")
                nc.sync.dma_start(
                    kvT_sb[:], kvT_d.ap().rearrange("(c p) t -> p c t", p=128)
                )

                def proj(W_sb, x_sb, b_sb, out_t, pfx):
                    for oc in range(NCH):
                        ps = pps.tile([128, L], F32, tag="psp", name=f"pp{pfx}_{oc}")
                        for di in range(NCH):
                            nc.tensor.matmul(
                                ps[:],
                                W_sb[:, di, oc * 128 : (oc + 1) * 128],
                                x_sb[:, di, :],
                                start=(di == 0),
                                stop=(di == NCH - 1),
                            )
                        nc.vector.tensor_scalar_add(
                            out_t[:, oc, :], ps[:], b_sb[:, oc : oc + 1]
                        )

                proj(WqS_sb, xqT_sb, bq_sb, qT_sb, "q")
                proj(Wk_sb, kvT_sb, bk_sb, kT_sb, "k")
                for tc4 in range(4):
                    for hf in range(2):
                        ps = pps.tile([128, 384], F32, tag="psv", name=f"ppv_{tc4}_{hf}")
                        for di in range(NCH):
                            nc.tensor.matmul(
                                ps[:],
                                kvT_sb[:, di, tc4 * 128 : (tc4 + 1) * 128],
                                Wv_sb[:, di, hf * 384 : (hf + 1) * 384],
                                start=(di == 0),
                                stop=(di == NCH - 1),
                            )
                        nc.vector.tensor_tensor(
                            v_sb[:, tc4, hf * 384 : (hf + 1) * 384],
                            ps[:],
                            bv_sb[:, hf * 384 : (hf + 1) * 384],
                            op=ADD,
                        )

            # ---- Phase 2: all-gather the bias across the 8 cores ----
            nc.gpsimd.collective_compute(
                "AllGather",
                mybir.AluOpType.bypass,
                replica_groups=[list(range(NCORES))],
                ins=[bias_shard[:].opt()],
                outs=[bias_full[:].opt()],
            )
            if dbg:
                nc.sync.dma_start(dbg_bfull[:, :], bias_full[:])

            # ---- Phase 3b: logits + softmax + AV per head ----
            bias_view = bias_full[:].rearrange("(q h) k -> q h k", h=H)
            with (
                tc.tile_pool(name="lps", bufs=2, space="PSUM") as lps,
                tc.tile_pool(name="trps", bufs=2, space="PSUM") as trps,
                tc.tile_pool(name="avps", bufs=2, space="PSUM") as avps,
                tc.tile_pool(name="battn", bufs=3) as battn,
                tc.tile_pool(name="bexp", bufs=2) as bexp,
                tc.tile_pool(name="bsm", bufs=4) as bsm,
                tc.tile_pool(name="bxp", bufs=2) as bxp,
            ):
                for h in range(H):
                    po = (h % 2) * DH
                    ch = h // 2
                    hs = slice(po, po + DH)
                    expT = bxp.tile([128, 4, L], F32R, tag="expT", name=f"expT_{h}")
                    for qc in range(4):
                        cs = slice(qc * 128, (qc + 1) * 128)
                        ps_l = lps.tile([128, L], F32, tag="lg", name=f"pl_{h}_{qc}")
                        nc.tensor.matmul(
                            ps_l[:],
                            qT_sb[hs, ch, cs],
                            kT_sb[hs, ch, :],
                            start=True,
                            stop=True,
                        )
                        bias_t = battn.tile(
                            [128, L], F32, tag="biast", name=f"bt_{h}_{qc}"
                        )
                        nc.sync.dma_start(
                            bias_t[:], bias_view[qc * 128 : (qc + 1) * 128, h, :]
                        )
                        lsb = battn.tile([128, L], F32, tag="lsb", name=f"ls_{h}_{qc}")
                        nc.vector.tensor_tensor(lsb[:], ps_l[:], bias_t[:], op=ADD)
                        exp_t = bexp.tile([128, L], F32, tag="exp", name=f"ex_{h}_{qc}")
                        sums = bsm.tile([128, 1], F32, tag="sums", name=f"sm_{h}_{qc}")
                        nc.scalar.activation(
                            exp_t[:], lsb[:], AF.Exp, accum_out=sums[:]
                        )
                        rc = bsm.tile([128, 1], F32, tag="rc", name=f"rc_{h}_{qc}")
                        nc.vector.reciprocal(rc[:], sums[:])
                        exp_s = bexp.tile(
                            [128, L], F32, tag="exps", name=f"exs_{h}_{qc}"
                        )
                        nc.vector.tensor_scalar_mul(exp_s[:], exp_t[:], rc[:])
                        for kc in range(4):
                            tr = trps.tile(
                                [128, 128], F32, tag="tr", name=f"tr_{h}_{qc}_{kc}"
                            )
                            nc.tensor.transpose(
                                tr[:], exp_s[:, kc * 128 : (kc + 1) * 128], ident[:]
                            )
                            nc.vector.tensor_copy(
                                expT[:, kc, qc * 128 : (qc + 1) * 128], tr[:]
                            )
                    ps_av = avps.tile([DH, L], F32, tag="av", name=f"av_{h}")
                    for kc in range(4):
                        nc.tensor.matmul(
                            ps_av[:],
                            v_sb[:, kc, h * DH : (h + 1) * DH],
                            expT[:, kc, :],
                            start=(kc == 0),
                            stop=(kc == 3),
                        )
                    nc.vector.tensor_copy(attnT[:, h, :], ps_av[:])

                # ---- Phase 3c: output projection (f32r) ----
                with tc.tile_pool(name="ops", bufs=2, space="PSUM") as ops:
                    for tc4 in range(4):
                        out_sb = battn.tile([128, D], F32, tag="osb", name=f"osb_{tc4}")
                        for hf in range(2):
                            ps_o = ops.tile(
                                [128, 384], F32, tag="pso", name=f"pso_{tc4}_{hf}"
                            )
                            sl = slice(hf * 384, (hf + 1) * 384)
                            for h2 in range(H):
                                nc.tensor.matmul(
                                    ps_o[:],
                                    attnT[:, h2, tc4 * 128 : (tc4 + 1) * 128],
                                    Wo_sb[:, h2, sl],
                                    start=(h2 == 0),
                                    stop=(h2 == H - 1),
                                )
                            nc.vector.tensor_tensor(
                                out_sb[:, sl], ps_o[:], bo_sb[:, sl], op=ADD
                            )
                        nc.sync.dma_start(
                            out_d[tc4 * 128 : (tc4 + 1) * 128, :], out_sb[:]
                        )

    nc.compile()
    return nc


def _get_nc():
    if "nc" not in _CACHE:
        _CACHE["nc"] = _build()
    return _CACHE["nc"]


def _hi_lo(a, dt):
    hi = a.astype(dt)
    lo = (a - hi.astype(np.float32)).astype(dt)
    return hi, lo


def kernel(
    query,
    key_value,
    query_coords,
    key_coords,
    Wq,
    bq,
    Wk,
    bk,
    Wv,
    bv,
    Wo,
    bo,
    W1,
    b1,
    W2,
    b2,
):
    import ml_dtypes

    query = np.asarray(query, np.float32)
    key_value = np.asarray(key_value, np.float32)
    query_coords = np.asarray(query_coords, np.float32)
    key_coords = np.asarray(key_coords, np.float32)

    def chunked(w, dt=np.float32):  # [768, X] -> [128, 6, X]
        w = np.asarray(w, dt)
        return np.ascontiguousarray(w.reshape(NCH, 128, -1).transpose(1, 0, 2))

    def pchunk(b):  # [768] -> [128, 6]
        return np.ascontiguousarray(np.asarray(b, np.float32).reshape(NCH, 128).T)

    WqS = chunked(np.asarray(Wq, np.float32) * np.float32(SCALE))
    Wk_l = chunked(Wk)
    Wv_l = chunked(Wv)
    Wo_l = np.ascontiguousarray(
        np.asarray(Wo, np.float32).reshape(H, DH, D).transpose(1, 0, 2)
    )
    W2hi, W2lo = _hi_lo(np.asarray(W2, np.float32), np.float16)
    W2h_l = chunked(W2hi, np.float16)
    W2l_l = chunked(W2lo, np.float16)
    W1f = np.asarray(W1, np.float32)
    W1hi, W1lo = _hi_lo(W1f, ml_dtypes.bfloat16)
    W1P = np.zeros((128, D), ml_dtypes.bfloat16)
    W1P[0:6] = W1hi
    W1P[6:12] = W1hi
    W1P[12:18] = W1lo
    W1P[18:24] = W1lo
    bqS = pchunk(np.asarray(bq, np.float32) * np.float32(SCALE))
    bk_l = pchunk(bk)
    b1_l = pchunk(b1)
    b2_l = np.ascontiguousarray(np.asarray(b2, np.float32).reshape(H, 1))
    bv_b = np.ascontiguousarray(np.broadcast_to(np.asarray(bv, np.float32), (128, D)))
    bo_b = np.ascontiguousarray(np.broadcast_to(np.asarray(bo, np.float32), (128, D)))

    in_maps = []
    for c in range(NCORES):
        qs = slice(c * QS, (c + 1) * QS)
        delta = query_coords[qs, None, :] - key_coords[None, :, :]
        rel = np.concatenate([delta, np.abs(delta), np.square(delta)], axis=-1)
        relT = rel.reshape(QS * L, 6).T
        rhi, rlo = _hi_lo(relT, ml_dtypes.bfloat16)
        relP = np.zeros((128, QS * L), ml_dtypes.bfloat16)
        relP[0:6] = rhi
        relP[6:12] = rlo
        relP[12:18] = rhi
        relP[18:24] = rlo
        in_maps.append(
            {
                "xqT": np.ascontiguousarray(query[c].T),
                "kvT": np.ascontiguousarray(key_value[c].T),
                "relP": relP,
                "WqS": WqS,
                "Wk": Wk_l,
                "Wv": Wv_l,
                "Wo": Wo_l,
                "W1P": W1P,
                "W2h": W2h_l,
                "W2l": W2l_l,
                "bqS": bqS,
                "bk": bk_l,
                "b1": b1_l,
                "b2": b2_l,
                "bvb": bv_b,
                "bob": bo_b,
            }
        )

    nc = _get_nc()
    res = bass_utils.run_bass_kernel_spmd(nc, in_maps, core_ids=list(range(NCORES)))
    out = np.stack([res.results[c]["out"] for c in range(NCORES)], axis=0)
    return out.astype(np.float32)
